# revision 1
# baseline (speedup 1.0000x reference)
"""Maxwell viscoelastic model (linear recurrence scan) on 8 Trainium2 NeuronCores.

Math (per trajectory, T timesteps):
    a_n = 1 - k*dt_n              (k = E/eta = 2)
    b_n = k*dt_n*eps_n
    gamma_n = a_n*gamma_{n-1} + b_n,  gamma_0 = 0
    sigma_n = (E_inf + E)*eps_n - E*gamma_n = 2.5*eps_n - 2*gamma_n

Kernel strategy: shard the batch (4096 trajectories) across 8 cores (512
each) — the recurrence is independent per trajectory, so pure data
parallelism.  Per core, 4 tiles of [128 partitions x 4096 timesteps], cut
into CH time-chunks that stream through a software pipeline.  The
recurrence runs on the DVE tensor_tensor_scan instruction:
    state = (data0 * state) + data1   per partition, along the free dim.
We scan g_n = a_n*g_{n-1} + (-E*b_n) so g = -E*gamma directly, then
sigma = (eps * 2.5) + g in one scalar_tensor_tensor op.

Engine split (the DVE's second SBUF read port is the one shared with
GpSimd, so every 2-input DVE op with both operands in SBUF locks GpSimd
out; routing one operand of each DVE op through PSUM frees that port):
  SYNC   loads xt chunks (HWDGE qSPDynamicHW ring)
  ACT    a = 1 - k*dt  -> PSUM, and issues output stores (qActDynamicHW)
  POOL   bneg = (dt * -E*k) * eps -> SBUF   (runs concurrently with DVE)
  DVE    scan(a[PSUM], bneg[SBUF]) -> g[PSUM]; sigma(eps[SBUF], g[PSUM])

Raw bass (no TileContext): the Tile scheduler attaches semaphore waits
directly to instructions and overflows the tiny ISA sync-wait budgets
(S2S2D2_STT takes a single wait; the tail Drain takes few). With raw bass
every wait is a standalone instruction and the pipeline is explicit.
DMA completion uses one semaphore per (buffer slot, chunk): two DMAs on
one ring can complete out of order, so a shared counter cannot tell which
transfer finished.
"""

import numpy as np

import concourse.bass as bass
import concourse.mybir as mybir
from concourse.bass_utils import run_bass_kernel_spmd

E = 2.0
ETA = 1.0
E_INFTY = 0.5
K = E / ETA                  # 2.0
NEG_EK = -(E * K)            # -4.0: scan data1 scale so the scan outputs -E*gamma
SIG_EPS = E_INFTY + E        # 2.5

N_CORES = 8
P = 128                      # SBUF partitions
CH = 4                       # time chunks per tile
XT_BUFS = 3                  # xt ring depth


def build_nc(b_shard: int, t_len: int) -> bass.Bass:
    nc = bass.Bass()
    x = nc.dram_tensor("x", [b_shard, t_len, 2], mybir.dt.float32, kind="ExternalInput")
    y = nc.dram_tensor("y", [b_shard, t_len], mybir.dt.float32, kind="ExternalOutput")
    n_tiles = b_shard // P
    assert n_tiles * P == b_shard
    assert t_len % CH == 0
    L = t_len // CH

    xr = x.rearrange("(n p) t c -> n p t c", p=P)   # [n_tiles, 128, T, 2]
    yr = y.rearrange("(n p) t -> n p t", p=P)       # [n_tiles, 128, T]
    f32 = mybir.dt.float32
    mult = mybir.AluOpType.mult
    add = mybir.AluOpType.add

    def cs(c):
        return slice(c * L, (c + 1) * L)

    with (
        nc.sbuf_tensor("xt0", [P, t_len, 2], f32) as xt0,
        nc.sbuf_tensor("xt1", [P, t_len, 2], f32) as xt1,
        nc.sbuf_tensor("xt2", [P, t_len, 2], f32) as xt2,
        nc.sbuf_tensor("bneg0", [P, L], f32) as bneg0,
        nc.sbuf_tensor("bneg1", [P, L], f32) as bneg1,
        nc.sbuf_tensor("e40", [P, L], f32) as e40,
        nc.sbuf_tensor("e41", [P, L], f32) as e41,
        nc.sbuf_tensor("sig0", [P, t_len], f32) as sig0,
        nc.sbuf_tensor("sig1", [P, t_len], f32) as sig1,
        nc.psum_tensor("pa0", [P, L], f32) as pa0,
        nc.psum_tensor("pa1", [P, L], f32) as pa1,
        nc.psum_tensor("pg0", [P, L], f32) as pg0,
        nc.psum_tensor("pg1", [P, L], f32) as pg1,
        nc.semaphore("act_a") as act_a,        # +1 per a chunk (ACT)
        nc.semaphore("act_e") as act_e,        # +1 per e4 chunk (ACT)
        nc.semaphore("pool_seq") as pool_seq,  # +1 per POOL instruction
        nc.semaphore("dve_seq") as dve_seq,    # +1 per DVE instruction
        nc.Block(no_gpsimd_drain=True) as block,
    ):
        sem_in = [
            [nc.alloc_semaphore(f"in{s}_{c}") for c in range(CH)]
            for s in range(XT_BUFS)
        ]
        sem_out = [[nc.alloc_semaphore(f"out{s}_{c}") for c in range(CH)] for s in range(2)]
        xt = [xt0, xt1, xt2]
        bneg = [bneg0, bneg1]
        e4 = [e40, e41]
        sig = [sig0, sig1]
        pa = [pa0, pa1]
        pg = [pg0, pg1]
        # q = CH*i + c. DVE: 2 instrs per chunk (scan -> 2q+1, sigma -> 2q+2).
        # POOL: 1 instr per chunk (bneg -> q+1). ACT: 1 a per chunk (act_a -> q+1).

        @block.sync
        def _(sync):
            for i in range(n_tiles):
                for c in range(CH):
                    if i >= XT_BUFS:
                        # xt slot chunk reuse: sigma(i-XT_BUFS, c) transitively
                        # implies every reader of that chunk finished.
                        sync.wait_ge(dve_seq, 2 * (CH * (i - XT_BUFS) + c) + 2)
                    sync.dma_start(
                        xt[i % XT_BUFS][:, cs(c), :], xr[i][:, cs(c), :]
                    ).then_inc(sem_in[i % XT_BUFS][c], 16)

        @block.gpsimd
        def _(gpsimd):
            for i in range(n_tiles):
                for c in range(CH):
                    q = CH * i + c
                    dtv = xt[i % XT_BUFS][:, cs(c), 1]
                    gpsimd.wait_ge(sem_in[i % XT_BUFS][c], 16 * (i // XT_BUFS + 1))
                    gpsimd.wait_ge(act_e, q + 1)   # e4(q) ready
                    if q >= 2:
                        # bneg slot WAR: scan(q-2) was the last reader.
                        gpsimd.wait_ge(dve_seq, 2 * (q - 2) + 1)
                    # bneg = dt * (-E*K * eps)   (TensorScalarPtr is not legal
                    # on Pool, so the -E*K scale rides on ACT's e4 pass)
                    gpsimd.tensor_tensor(
                        bneg[q % 2][:], dtv, e4[q % 2][:], mult,
                    ).then_inc(pool_seq, 1)

        @block.scalar
        def _(scalar):
            def store(k):
                i, c = divmod(k, CH)
                scalar.wait_ge(dve_seq, 2 * k + 2)   # sigma(k) complete
                scalar.dma_start(
                    yr[i][:, cs(c)], sig[i % 2][:, cs(c)]
                ).then_inc(sem_out[i % 2][c], 16)

            for i in range(n_tiles):
                for c in range(CH):
                    q = CH * i + c
                    scalar.wait_ge(sem_in[i % XT_BUFS][c], 16 * (i // XT_BUFS + 1))
                    if q >= 2:
                        # a slot WAR: scan(q-2) read it.
                        scalar.wait_ge(dve_seq, 2 * (q - 2) + 1)
                    # a = Copy(dt * -K + 1) -> PSUM
                    scalar.activation(
                        pa[q % 2][:], xt[i % XT_BUFS][:, cs(c), 1],
                        mybir.ActivationFunctionType.Copy,
                        bias=1.0, scale=-K,
                    ).then_inc(act_a, 1)
                    if q >= 2:
                        # e4 slot WAR: bneg(q-2) read it.
                        scalar.wait_ge(pool_seq, q - 1)
                    # e4 = Copy(eps * -E*K) -> SBUF (feeds POOL's bneg)
                    scalar.activation(
                        e4[q % 2][:], xt[i % XT_BUFS][:, cs(c), 0],
                        mybir.ActivationFunctionType.Copy,
                        bias=0.0, scale=NEG_EK,
                    ).then_inc(act_e, 1)
                    if q >= 1:
                        store(q - 1)
            store(CH * n_tiles - 1)
            for c in range(CH):
                scalar.wait_ge(sem_out[0][c], 16 * ((n_tiles + 1) // 2))
                if n_tiles >= 2:
                    scalar.wait_ge(sem_out[1][c], 16 * (n_tiles // 2))

        @block.vector
        def _(vector):
            for i in range(n_tiles):
                for c in range(CH):
                    q = CH * i + c
                    eps = xt[i % XT_BUFS][:, cs(c), 0]
                    vector.wait_ge(sem_in[i % XT_BUFS][c], 16 * (i // XT_BUFS + 1))
                    vector.wait_ge(act_a, q + 1)       # a(q) in PSUM
                    vector.wait_ge(pool_seq, q + 1)    # bneg(q) in SBUF
                    if q >= 1:
                        vector.wait_ge(dve_seq, 2 * q)  # sigma(q-1) complete
                    # g_n = a_n*g_{n-1} + bneg_n  ->  g = -E*gamma
                    # Chain across chunks: initial = last element of the
                    # previous chunk's g; fresh 0 at each tile's chunk 0.
                    init = 0.0 if c == 0 else pg[(q - 1) % 2][:, L - 1:L]
                    vector.tensor_tensor_scan(
                        pg[q % 2][:], pa[q % 2][:], bneg[q % 2][:], init, mult, add,
                    ).then_inc(dve_seq, 1)
                    if i >= 2:
                        # sig slot chunk reuse: store(i-2, c) completed.
                        vector.wait_ge(sem_out[i % 2][c], 16 * ((i - 2) // 2 + 1))
                    vector.wait_ge(dve_seq, 2 * q + 1)   # scan complete
                    # sigma = (eps * 2.5) + g
                    vector.scalar_tensor_tensor(
                        sig[i % 2][:, cs(c)], eps, SIG_EPS, pg[q % 2][:], mult, add,
                    ).then_inc(dve_seq, 1)

    return nc


_NC_CACHE: dict = {}


def _get_nc(b_shard: int, t_len: int) -> bass.Bass:
    key = (b_shard, t_len)
    if key not in _NC_CACHE:
        _NC_CACHE[key] = build_nc(b_shard, t_len)
    return _NC_CACHE[key]


def run(x: np.ndarray, trace: bool = False):
    """Run the sharded kernel; returns (full_output, BassKernelResults)."""
    b, t_len, c = x.shape
    assert c == 2 and b % N_CORES == 0
    b_shard = b // N_CORES
    x = np.ascontiguousarray(np.asarray(x, dtype=np.float32))
    shards = x.reshape(N_CORES, b_shard, t_len, 2)
    in_maps = [{"x": shards[i]} for i in range(N_CORES)]
    res = run_bass_kernel_spmd(
        _get_nc(b_shard, t_len), in_maps,
        core_ids=list(range(N_CORES)), trace=trace,
    )
    out = np.concatenate([r["y"] for r in res.results], axis=0)
    return out.reshape(b, t_len, 1), res


def kernel(x: np.ndarray) -> np.ndarray:
    out, _ = run(x, trace=False)
    return out



# revision 2
# speedup vs baseline: 1.4795x; 1.4795x over previous
"""Maxwell viscoelastic model (linear recurrence scan) on 8 Trainium2 NeuronCores.

Math (per trajectory, T timesteps):
    a_n = 1 - 2*dt_n
    gamma_n = a_n*gamma_{n-1} + 2*dt_n*eps_n,  gamma_0 = 0
    sigma_n = 2.5*eps_n - 2*gamma_n

Strategy: batch-shard 4096 trajectories across 8 cores (512 each); all I/O in
fp16 (the 2e-2 rel-err budget dwarfs fp16 quantization at ~6e-4), which halves
HBM traffic to ~12.6 MB/core and doubles DVE elementwise throughput.  The host
packs per-core tiles of [128 rows x (eps|dm4) chunk planes] where dm4 = -4*dt,
and unpacks the fp16 sigma output back to f32.

Per chunk q (L=2048 timesteps) the recurrence g = -2*gamma runs as
    g = a*g + (dm4*eps)        [tensor_tensor_scan]
    sigma = 2.5*eps + g
with the engine split (DVE's 2nd SBUF read port is shared with GpSimd, so the
scan keeps data0 in PSUM to leave that port free):
  SYNC   chunk loads + tile stores (one HWDGE ring)
  ACT    a = 1 + 0.5*dm4 -> PSUM f32;  e25 = 2.5*eps -> SBUF fp16
  POOL   bneg = dm4 * eps -> SBUF fp16   (GpSimd)
  DVE    scan(a[PSUM], bneg[SBUF]) -> g[SBUF fp16]; sigma = e25 + g (fp16 2x)

Raw bass (no TileContext): explicit semaphores, one per (buffer slot, chunk)
for DMA completion since two DMAs on one ring can complete out of order.
"""

import numpy as np

import concourse.bass as bass
import concourse.mybir as mybir
from concourse.bass_utils import run_bass_kernel_spmd

N_CORES = 8
P = 128                      # SBUF partitions
T = 4096                     # timesteps
CH = 2                       # chunks per tile
L = T // CH                  # 2048 timesteps per chunk
ROW = 2 * T                  # packed row: CH blocks of (eps[L] | dm4[L])
XT_BUFS = 3                  # xt tile ring depth

f32 = mybir.dt.float32
f16 = mybir.dt.float16


def build_nc(b_shard: int) -> bass.Bass:
    nc = bass.Bass()
    x = nc.dram_tensor("x", [b_shard, ROW], f16, kind="ExternalInput")
    y = nc.dram_tensor("y", [b_shard, T], f16, kind="ExternalOutput")
    n_tiles = b_shard // P
    assert n_tiles * P == b_shard

    xr = x.rearrange("(n p) f -> n p f", p=P)   # [n_tiles, 128, ROW]
    yr = y.rearrange("(n p) t -> n p t", p=P)   # [n_tiles, 128, T]
    mult = mybir.AluOpType.mult
    add = mybir.AluOpType.add
    Copy = mybir.ActivationFunctionType.Copy
    NQ = CH * n_tiles

    def eps_s(c):
        return slice(2 * L * c, 2 * L * c + L)

    def dm4_s(c):
        return slice(2 * L * c + L, 2 * L * c + 2 * L)

    def sig_s(c):
        return slice(L * c, L * (c + 1))

    with (
        nc.sbuf_tensor("xt0", [P, ROW], f16) as xt0,
        nc.sbuf_tensor("xt1", [P, ROW], f16) as xt1,
        nc.sbuf_tensor("xt2", [P, ROW], f16) as xt2,
        nc.sbuf_tensor("bneg0", [P, L], f16) as bneg0,
        nc.sbuf_tensor("bneg1", [P, L], f16) as bneg1,
        nc.sbuf_tensor("e250", [P, L], f16) as e250,
        nc.sbuf_tensor("e251", [P, L], f16) as e251,
        nc.sbuf_tensor("g0", [P, L], f16) as g0,
        nc.sbuf_tensor("g1", [P, L], f16) as g1,
        nc.sbuf_tensor("sig0", [P, T], f16) as sig0,
        nc.sbuf_tensor("sig1", [P, T], f16) as sig1,
        nc.psum_tensor("pa0", [P, L], f32) as pa0,
        nc.psum_tensor("pa1", [P, L], f32) as pa1,
        nc.semaphore("act_a") as act_a,        # +1 per a chunk (ACT)
        nc.semaphore("act_e") as act_e,        # +1 per e25 chunk (ACT)
        nc.semaphore("pool_seq") as pool_seq,  # +1 per bneg chunk (POOL)
        nc.semaphore("dve_seq") as dve_seq,    # +1 per DVE instruction
        nc.Block(no_gpsimd_drain=True) as block,
    ):
        sem_in = [
            [nc.alloc_semaphore(f"in{s}_{c}") for c in range(CH)]
            for s in range(XT_BUFS)
        ]
        sem_out = [nc.alloc_semaphore(f"out{s}") for s in range(2)]
        xt = [xt0, xt1, xt2]
        bneg = [bneg0, bneg1]
        e25 = [e250, e251]
        g = [g0, g1]
        sig = [sig0, sig1]
        pa = [pa0, pa1]
        # q = CH*i + c. DVE: 2 instrs per chunk (scan -> 2q+1, sigma -> 2q+2).

        @block.sync
        def _(sync):
            for i in range(n_tiles):
                for c in range(CH):
                    q = CH * i + c
                    if i >= XT_BUFS:
                        # xt slot chunk reuse: sigma(i-XT_BUFS, c) transitively
                        # implies every reader of that chunk finished (sigma
                        # follows scan on DVE; scan read bneg/a which read xt).
                        sync.wait_ge(dve_seq, 2 * (CH * (i - XT_BUFS) + c) + 2)
                    sync.dma_start(
                        xt[i % XT_BUFS][:, 2 * L * c : 2 * L * (c + 1)],
                        xr[i][:, 2 * L * c : 2 * L * (c + 1)],
                    ).then_inc(sem_in[i % XT_BUFS][c], 16)
                if i >= 2:
                    # store(i-2): sigma(i-2, CH-1) done -> whole sig tile ready
                    sync.wait_ge(dve_seq, 2 * (CH * (i - 2) + CH - 1) + 2)
                    sync.dma_start(yr[i - 2][:], sig[i % 2][:]).then_inc(
                        sem_out[i % 2], 16
                    )
            for i in range(n_tiles - 2, n_tiles):
                sync.wait_ge(dve_seq, 2 * (CH * i + CH - 1) + 2)
                sync.dma_start(yr[i][:], sig[i % 2][:]).then_inc(
                    sem_out[i % 2], 16
                )
            # drain: all stores complete
            sync.wait_ge(sem_out[0], 16 * ((n_tiles + 1) // 2))
            sync.wait_ge(sem_out[1], 16 * (n_tiles // 2))

        @block.scalar
        def _(scalar):
            for i in range(n_tiles):
                for c in range(CH):
                    q = CH * i + c
                    scalar.wait_ge(sem_in[i % XT_BUFS][c], 16 * (i // XT_BUFS + 1))
                    if q >= 2:
                        # pa slot WAR: scan(q-2) was the reader
                        scalar.wait_ge(dve_seq, 2 * (q - 2) + 1)
                    # a = 1 + 0.5*dm4 -> PSUM f32
                    scalar.activation(
                        pa[q % 2][:], xt[i % XT_BUFS][:, dm4_s(c)],
                        Copy, bias=1.0, scale=0.5,
                    ).then_inc(act_a, 1)
                    if q >= 2:
                        # e25 slot WAR: sigma(q-2) was the reader
                        scalar.wait_ge(dve_seq, 2 * (q - 2) + 2)
                    # e25 = 2.5*eps -> SBUF fp16
                    scalar.activation(
                        e25[q % 2][:], xt[i % XT_BUFS][:, eps_s(c)],
                        Copy, bias=0.0, scale=2.5,
                    ).then_inc(act_e, 1)

        @block.gpsimd
        def _(gpsimd):
            for i in range(n_tiles):
                for c in range(CH):
                    q = CH * i + c
                    gpsimd.wait_ge(sem_in[i % XT_BUFS][c], 16 * (i // XT_BUFS + 1))
                    if q >= 2:
                        # bneg slot WAR: scan(q-2) was the reader
                        gpsimd.wait_ge(dve_seq, 2 * (q - 2) + 1)
                    # bneg = dm4 * eps -> SBUF fp16
                    gpsimd.tensor_tensor(
                        bneg[q % 2][:],
                        xt[i % XT_BUFS][:, dm4_s(c)],
                        xt[i % XT_BUFS][:, eps_s(c)],
                        mult,
                    ).then_inc(pool_seq, 1)

        @block.vector
        def _(vector):
            for i in range(n_tiles):
                for c in range(CH):
                    q = CH * i + c
                    vector.wait_ge(act_a, q + 1)       # a(q) in PSUM
                    vector.wait_ge(pool_seq, q + 1)    # bneg(q) in SBUF
                    # g slot WAR: sigma(q-2) read g[q%2]; same-engine order.
                    # chunk chain: init = last element of previous chunk's g
                    init = 0.0 if c == 0 else g[(q - 1) % 2][:, L - 1 : L]
                    vector.tensor_tensor_scan(
                        g[q % 2][:], pa[q % 2][:], bneg[q % 2][:], init, mult, add,
                    ).then_inc(dve_seq, 1)
                    vector.wait_ge(act_e, q + 1)       # e25(q) in SBUF
                    if i >= 2 and c == 0:
                        # sig slot WAR: store(i-2) completed
                        vector.wait_ge(sem_out[i % 2], 16 * ((i - 2) // 2 + 1))
                    # sigma = e25 + g   (fp16 2x tensor_tensor)
                    vector.tensor_tensor(
                        sig[i % 2][:, sig_s(c)], e25[q % 2][:], g[q % 2][:], add,
                    ).then_inc(dve_seq, 1)

    return nc


_NC_CACHE: dict = {}


def _get_nc(b_shard: int) -> bass.Bass:
    if b_shard not in _NC_CACHE:
        _NC_CACHE[b_shard] = build_nc(b_shard)
    return _NC_CACHE[b_shard]


def _pack(x: np.ndarray) -> np.ndarray:
    """[B, T, 2] f32 -> [N_CORES, b_shard, ROW] fp16 packed (eps|dm4 chunks)."""
    b = x.shape[0]
    eps = x[:, :, 0].astype(np.float16)
    dm4 = (-4.0 * x[:, :, 1]).astype(np.float16)
    packed = np.empty((b, CH, 2, L), dtype=np.float16)
    packed[:, :, 0, :] = eps.reshape(b, CH, L)
    packed[:, :, 1, :] = dm4.reshape(b, CH, L)
    return packed.reshape(N_CORES, b // N_CORES, ROW)


def run(x: np.ndarray, trace: bool = False):
    """Run the sharded kernel; returns (full_output, BassKernelResults)."""
    b, t_len, c = x.shape
    assert c == 2 and t_len == T and b % N_CORES == 0
    b_shard = b // N_CORES
    shards = _pack(np.asarray(x, dtype=np.float32))
    in_maps = [{"x": np.ascontiguousarray(shards[i])} for i in range(N_CORES)]
    res = run_bass_kernel_spmd(
        _get_nc(b_shard), in_maps,
        core_ids=list(range(N_CORES)), trace=trace,
    )
    out = np.concatenate([r["y"] for r in res.results], axis=0)
    return out.astype(np.float32).reshape(b, t_len, 1), res


def kernel(x: np.ndarray) -> np.ndarray:
    out, _ = run(x, trace=False)
    return out


# revision 5
# speedup vs baseline: 1.4942x; 1.0100x over previous
"""Maxwell viscoelastic model (linear recurrence scan) on 8 Trainium2 NeuronCores.

Math (per trajectory, T timesteps):
    a_n = 1 - 2*dt_n
    gamma_n = a_n*gamma_{n-1} + 2*dt_n*eps_n,  gamma_0 = 0
    sigma_n = 2.5*eps_n - 2*gamma_n

Strategy: batch-shard 4096 trajectories across 8 cores (512 each); all I/O in
fp16 (the 2e-2 rel-err budget dwarfs fp16 quantization at ~1e-3), which halves
HBM traffic to ~12.6 MB/core and doubles DVE elementwise throughput.  The host
packs per-core tiles of [128 rows x (eps plane | dm4 plane)] where dm4 = -4*dt,
and unpacks the fp16 sigma output back to f32.

Per chunk q (L=2048 timesteps) the recurrence g = -2*gamma runs as
    g = a*g + (dm4*eps)        [tensor_tensor_scan, ~2 cycles/elem]
    sigma = 2.5*eps + g        [fp16 2x tensor_tensor]
with the engine split (DVE's 2nd SBUF read port is shared with GpSimd, so the
scan keeps data0 in PSUM to leave that port free for GpSimd):
  SYNC   chunk loads + chunk stores (one HWDGE ring)
  ACT    e25 = 2.5*eps -> SBUF fp16;  a = 1 + 0.5*dm4 -> PSUM f32
  POOL   bneg = dm4 * eps -> SBUF fp16   (GpSimd)
  DVE    scan(a[PSUM], bneg[SBUF]) -> g[SBUF fp16]; sigma = e25 + g

The first chunk of tile 0 is processed in 4 sub-pieces of 512 timesteps so the
first scan starts ~8us earlier (load->bneg->scan ramp on 256KB instead of 1MB).

Raw bass (no TileContext): explicit semaphores, one per (buffer slot, chunk)
for DMA completion since two DMAs on one ring can complete out of order.
"""

import numpy as np

import concourse.bass as bass
import concourse.mybir as mybir
from concourse.bass_utils import run_bass_kernel_spmd

N_CORES = 8
P = 128                      # SBUF partitions
T = 4096                     # timesteps
CH = 2                       # chunks per tile
L = T // CH                  # 2048 timesteps per chunk
NP0 = 4                      # ramp sub-pieces for tile 0 chunk 0
L0 = L // NP0                # 512
ROW = 2 * T                  # packed row: eps[T] | dm4[T]
XT_BUFS = 3                  # xt tile ring depth

f32 = mybir.dt.float32
f16 = mybir.dt.float16


def build_nc(b_shard: int) -> bass.Bass:
    nc = bass.Bass()
    x = nc.dram_tensor("x", [b_shard, ROW], f16, kind="ExternalInput")
    y = nc.dram_tensor("y", [b_shard, T], f16, kind="ExternalOutput")
    n_tiles = b_shard // P
    assert n_tiles * P == b_shard

    xr = x.rearrange("(n p) f -> n p f", p=P)   # [n_tiles, 128, ROW]
    yr = y.rearrange("(n p) t -> n p t", p=P)   # [n_tiles, 128, T]
    mult = mybir.AluOpType.mult
    add = mybir.AluOpType.add
    Copy = mybir.ActivationFunctionType.Copy

    def eps_s(c, p0=0, p1=None):
        base = L * c
        return slice(base + (p0 or 0) * L0, base + (NP0 if p1 is None else p1) * L0)

    def dm4_s(c, p0=0, p1=None):
        base = T + L * c
        return slice(base + (p0 or 0) * L0, base + (NP0 if p1 is None else p1) * L0)

    def chunk_s(c, p0=0, p1=None):
        base = L * c
        return slice(base + (p0 or 0) * L0, base + (NP0 if p1 is None else p1) * L0)

    with (
        nc.sbuf_tensor("xt0", [P, ROW], f16) as xt0,
        nc.sbuf_tensor("xt1", [P, ROW], f16) as xt1,
        nc.sbuf_tensor("xt2", [P, ROW], f16) as xt2,
        nc.sbuf_tensor("bneg0", [P, L], f16) as bneg0,
        nc.sbuf_tensor("bneg1", [P, L], f16) as bneg1,
        nc.sbuf_tensor("e250", [P, L], f16) as e250,
        nc.sbuf_tensor("e251", [P, L], f16) as e251,
        nc.sbuf_tensor("g0", [P, L], f16) as g0,
        nc.sbuf_tensor("g1", [P, L], f16) as g1,
        nc.sbuf_tensor("sig0", [P, T], f16) as sig0,
        nc.sbuf_tensor("sig1", [P, T], f16) as sig1,
        nc.psum_tensor("pa0", [P, L], f32) as pa0,
        nc.psum_tensor("pa1", [P, L], f32) as pa1,
        nc.semaphore("act_seq") as act_seq,    # +1 per ACT op (e25 -> 2q+1, a -> 2q+2)
        nc.semaphore("pool_seq") as pool_seq,  # +1 per bneg chunk (POOL)
        nc.semaphore("dve_seq") as dve_seq,    # +1 per sigma (DVE); sigma(q) -> q+1
        nc.Block(no_gpsimd_drain=True) as block,
    ):
        sem_in = [
            [nc.alloc_semaphore(f"in{s}_{c}") for c in range(CH)]
            for s in range(XT_BUFS)
        ]
        # ramp pieces of tile 0 chunk 0 get their own completion sems
        sem_p = [nc.alloc_semaphore(f"inp{k}") for k in range(1, NP0)]
        sem_out = [
            [nc.alloc_semaphore(f"out{s}_{c}") for c in range(CH)] for s in range(2)
        ]
        xt = [xt0, xt1, xt2]
        bneg = [bneg0, bneg1]
        e25 = [e250, e251]
        g = [g0, g1]
        sig = [sig0, sig1]
        pa = [pa0, pa1]
        NQ = CH * n_tiles

        @block.sync
        def _(sync):
            # tile 0 chunk 0 in pieces (eps piece + dm4 piece per load pair)
            for k in range(NP0):
                s = sem_in[0][0] if k == 0 else sem_p[k - 1]
                sync.dma_start(xt[0][:, eps_s(0, k, k + 1)],
                               xr[0][:, eps_s(0, k, k + 1)]).then_inc(s, 16)
                sync.dma_start(xt[0][:, dm4_s(0, k, k + 1)],
                               xr[0][:, dm4_s(0, k, k + 1)]).then_inc(s, 16)
            for i in range(n_tiles):
                for c in range(CH):
                    if i == 0 and c == 0:
                        continue
                    q = CH * i + c
                    if i >= XT_BUFS:
                        # xt slot chunk reuse: sigma(i-XT_BUFS, c) transitively
                        # implies every reader of that chunk finished.
                        sync.wait_ge(dve_seq, CH * (i - XT_BUFS) + c + 1)
                    sync.dma_start(
                        xt[i % XT_BUFS][:, eps_s(c)], xr[i][:, eps_s(c)]
                    ).then_inc(sem_in[i % XT_BUFS][c], 16)
                    sync.dma_start(
                        xt[i % XT_BUFS][:, dm4_s(c)], xr[i][:, dm4_s(c)]
                    ).then_inc(sem_in[i % XT_BUFS][c], 16)
            for i in range(n_tiles):
                for c in range(CH):
                    q = CH * i + c
                    sync.wait_ge(dve_seq, q + 1)     # sigma(q) done
                    sync.dma_start(
                        yr[i][:, chunk_s(c)], sig[i % 2][:, chunk_s(c)]
                    ).then_inc(sem_out[i % 2][c], 16)
            for c in range(CH):
                sync.wait_ge(sem_out[0][c], 16 * ((n_tiles + 1) // 2))
                if n_tiles >= 2:
                    sync.wait_ge(sem_out[1][c], 16 * (n_tiles // 2))

        @block.scalar
        def _(scalar):
            for i in range(n_tiles):
                for c in range(CH):
                    q = CH * i + c
                    pieces = (
                        [(k, k + 1) for k in range(NP0)]
                        if (i == 0 and c == 0) else [(0, NP0)]
                    )
                    for pi, (k0, k1) in enumerate(pieces):
                        if i == 0 and c == 0:
                            s = sem_in[0][0] if k0 == 0 else sem_p[k0 - 1]
                            scalar.wait_ge(s, 32)
                        elif pi == 0:
                            scalar.wait_ge(
                                sem_in[i % XT_BUFS][c], 32 * (i // XT_BUFS + 1)
                            )
                        if q >= 2 and pi == 0:
                            # e25/pa slot WAR: sigma(q-2) read e25 and followed
                            # scan(q-2), the pa reader.
                            scalar.wait_ge(dve_seq, q - 1)
                        # e25 = 2.5*eps -> SBUF fp16   (before a: scan(q)'s
                        # act_seq wait then also covers sigma(q)'s e25 dep)
                        scalar.activation(
                            e25[q % 2][:, L0 * k0 : L0 * k1],
                            xt[i % XT_BUFS][:, eps_s(c, k0, k1)],
                            Copy, bias=0.0, scale=2.5,
                        ).then_inc(act_seq, 1)
                        # a = 1 + 0.5*dm4 -> PSUM f32
                        scalar.activation(
                            pa[q % 2][:, L0 * k0 : L0 * k1],
                            xt[i % XT_BUFS][:, dm4_s(c, k0, k1)],
                            Copy, bias=1.0, scale=0.5,
                        ).then_inc(act_seq, 1)

        @block.gpsimd
        def _(gpsimd):
            for i in range(n_tiles):
                for c in range(CH):
                    q = CH * i + c
                    pieces = (
                        [(k, k + 1) for k in range(NP0)]
                        if (i == 0 and c == 0) else [(0, NP0)]
                    )
                    for pi, (k0, k1) in enumerate(pieces):
                        if i == 0 and c == 0:
                            s = sem_in[0][0] if k0 == 0 else sem_p[k0 - 1]
                            gpsimd.wait_ge(s, 32)
                        elif pi == 0:
                            gpsimd.wait_ge(
                                sem_in[i % XT_BUFS][c], 32 * (i // XT_BUFS + 1)
                            )
                        if q >= 2 and pi == 0:
                            # bneg slot WAR: sigma(q-2) followed scan(q-2)
                            gpsimd.wait_ge(dve_seq, q - 1)
                        # bneg = dm4 * eps -> SBUF fp16
                        gpsimd.tensor_tensor(
                            bneg[q % 2][:, L0 * k0 : L0 * k1],
                            xt[i % XT_BUFS][:, dm4_s(c, k0, k1)],
                            xt[i % XT_BUFS][:, eps_s(c, k0, k1)],
                            mult,
                        ).then_inc(pool_seq, 1)

        @block.vector
        def _(vector):
            # ACT op count before chunk q completes: tile0-chunk0 counts 2*NP0
            def act_done(q):
                return 2 * NP0 + 2 * (q - 1) + 2 if q >= 1 else 2 * NP0

            def pool_done(q):
                return NP0 + q if q >= 1 else NP0

            for i in range(n_tiles):
                for c in range(CH):
                    q = CH * i + c
                    pieces = (
                        [(k, k + 1) for k in range(NP0)]
                        if (i == 0 and c == 0) else [(0, NP0)]
                    )
                    np_q = len(pieces)
                    for pi, (k0, k1) in enumerate(pieces):
                        base_a = act_done(q - 1) if q >= 1 else 0
                        base_p = pool_done(q - 1) if q >= 1 else 0
                        vector.wait_ge(act_seq, base_a + 2 * (pi + 1))
                        vector.wait_ge(pool_seq, base_p + pi + 1)
                        if k0 == 0:
                            init = (
                                0.0 if c == 0
                                else g[(q - 1) % 2][:, L - 1 : L]
                            )
                        else:
                            init = g[q % 2][:, L0 * k0 - 1 : L0 * k0]
                        # g = a*g + bneg  (g slot WAR: sigma(q-2) preceded on DVE)
                        vector.tensor_tensor_scan(
                            g[q % 2][:, L0 * k0 : L0 * k1],
                            pa[q % 2][:, L0 * k0 : L0 * k1],
                            bneg[q % 2][:, L0 * k0 : L0 * k1],
                            init, mult, add,
                        )
                        if np_q > 1:
                            # per-piece sigma; doubles as the intervening op
                            # that keeps the next scan's `initial` read from
                            # racing this scan's in-flight last write.
                            tt = vector.tensor_tensor(
                                sig[i % 2][:, chunk_s(c, k0, k1)],
                                e25[q % 2][:, L0 * k0 : L0 * k1],
                                g[q % 2][:, L0 * k0 : L0 * k1],
                                add,
                            )
                            if pi == np_q - 1:
                                tt.then_inc(dve_seq, 1)
                    if np_q > 1:
                        continue
                    if i >= 2:
                        # sig slot WAR: store(i-2, c) completed
                        vector.wait_ge(
                            sem_out[i % 2][c], 16 * ((i - 2) // 2 + 1)
                        )
                    # sigma = e25 + g   (fp16 2x tensor_tensor)
                    vector.tensor_tensor(
                        sig[i % 2][:, chunk_s(c)], e25[q % 2][:], g[q % 2][:], add,
                    ).then_inc(dve_seq, 1)

    return nc


_NC_CACHE: dict = {}


def _get_nc(b_shard: int) -> bass.Bass:
    if b_shard not in _NC_CACHE:
        _NC_CACHE[b_shard] = build_nc(b_shard)
    return _NC_CACHE[b_shard]


def _pack(x: np.ndarray) -> np.ndarray:
    """[B, T, 2] f32 -> [N_CORES, b_shard, ROW] fp16 (eps plane | dm4 plane)."""
    b = x.shape[0]
    packed = np.empty((b, 2, T), dtype=np.float16)
    packed[:, 0, :] = x[:, :, 0].astype(np.float16)
    packed[:, 1, :] = (-4.0 * x[:, :, 1]).astype(np.float16)
    return packed.reshape(N_CORES, b // N_CORES, ROW)


def run(x: np.ndarray, trace: bool = False):
    """Run the sharded kernel; returns (full_output, BassKernelResults)."""
    b, t_len, c = x.shape
    assert c == 2 and t_len == T and b % N_CORES == 0
    b_shard = b // N_CORES
    shards = _pack(np.asarray(x, dtype=np.float32))
    in_maps = [{"x": np.ascontiguousarray(shards[i])} for i in range(N_CORES)]
    res = run_bass_kernel_spmd(
        _get_nc(b_shard), in_maps,
        core_ids=list(range(N_CORES)), trace=trace,
    )
    out = np.concatenate([r["y"] for r in res.results], axis=0)
    return out.astype(np.float32).reshape(b, t_len, 1), res


def kernel(x: np.ndarray) -> np.ndarray:
    out, _ = run(x, trace=False)
    return out
